# revision 4
# baseline (speedup 1.0000x reference)
"""CapsuleNet dynamic-routing kernel for 8 Trainium2 NeuronCores.

Sharding: input capsules (I=2048) split 256-per-core; every core holds the
full batch (B=128).  The only cross-core exchange is an AllReduce of the
partial capsule sums s (32x2048 fp32 = 256KB) once per routing iteration.

x_hat is never materialized.  Out-capsules are processed in PAIRS (o = 2p+o2)
so all matmul operands sit at base partition 0 with K/M = 32 (the PE only
allows operand base partitions {0,32,64}):

  t[b,o,(i,k)] = sum_d v[b,o,d] W[o,i,d,k]     PE: lhsT = v^T pair [32,b],
                                               rhs = W2P pair-block-diag
  L[b,o,i]   = sum_k x[b,i,k] t[b,o,i,k]       DVE mult + GPSIMD add-tree
  B += L ; c = softmax_o(B)                    ACT exp + DVE tree/reciprocal
  y_o[(ik),b] = c[b,i] x[b,(k,i)]^T            DVE (chunks are (k,i)-major)
  s^T[(o2,d),(p,b)] = sum_ik W1P y_o           PE: lhsT = W1P pair-slot cols
  AllReduce(s^T) ; v = squash(s^T)             E4/E5 ones-matmuls for the
                                               partition-group d-reduction
"""

import numpy as np
import ml_dtypes

import concourse.bass as bass
import concourse.mybir as mybir
import concourse.tile as tile
from concourse import bacc
from concourse.bass_utils import run_bass_kernel_spmd

BF16 = mybir.dt.bfloat16
F32 = mybir.dt.float32
AF = mybir.ActivationFunctionType
OP = mybir.AluOpType

B = 128          # batch
KC = 8           # in capsule dim (conv channels)
I_FULL = 2048    # in capsules total
O = 32           # out capsules
D = 16           # out capsule dim
NP = 16          # out-capsule pairs
NCORES = 8
IL = I_FULL // NCORES           # 256 in-capsules per core
IK = IL * KC                    # 2048 local (i,k) elements
NQ = IK // 128                  # 16 partition chunks of the (k,i) axis
EPS = 1e-8

_CACHE: dict = {}


def _squash_pair(nc, wp, ocp, pt, pv, s_ps, s_all, e4_sb, e5_sb, dram_pool,
                 vT2, out_sb, scale0, final):
    """AllReduce + squash in [(o2,d), (p,b)] layout, processed in 512-column
    chunks so the per-partition scratch stays tiny.  Small scratch tiles are
    drawn from the o-loop pool (its slots are idle during the squash)."""
    s_sb = wp.tile([32, NP * B], F32, tag="s_sb")
    if scale0 != 1.0:
        nc.scalar.mul(s_sb[:, :], s_ps[:, :, :].rearrange("a p b -> a (p b)"),
                      scale0)
    else:
        nc.scalar.copy(s_sb[:, :],
                       s_ps[:, :, :].rearrange("a p b -> a (p b)"))

    cc_in = dram_pool.tile([32, NP * B], F32, tag="cc_in")
    cc_out = dram_pool.tile([32, NP * B], F32, tag="cc_out")
    nc.sync.dma_start(cc_in[:, :], s_sb[:, :])
    nc.gpsimd.collective_compute(
        "AllReduce",
        OP.add,
        replica_groups=[list(range(NCORES))],
        ins=[cc_in.opt()],
        outs=[cc_out.opt()],
    )
    nc.sync.dma_start(s_all[:, :], cc_out[:, :])

    sq2 = wp.tile([32, NP * B], F32, tag="sq2_out")
    nc.vector.tensor_tensor(sq2[:, :], s_all[:, :], s_all[:, :], OP.mult)

    for n in range(4):
        cs = slice(n * 512, (n + 1) * 512)
        # nsq[o2, (p,b)] = sum_d s^2 over the two 16-partition groups
        nsq = pt.tile([2, 512], F32, tag="t_ps")
        nc.tensor.matmul(nsq[:, :], lhsT=e4_sb[:, :], rhs=sq2[:, cs],
                         start=True, stop=True)
        rt = ocp.tile([2, 512], F32, tag="t_sb")
        nc.scalar.sqrt(rt[:, :], nsq[:, :])
        if final:
            num = ocp.tile([2, 512], F32, tag="lt")
            nc.vector.tensor_tensor(num[:, :], nsq[:, :], rt[:, :], OP.mult)
        nc.vector.tensor_scalar_add(rt[:, :], rt[:, :], EPS)
        den = ocp.tile([2, 512], F32, tag="z")
        nc.vector.scalar_tensor_tensor(       # (nsq + 1) * (r + eps)
            den[:, :], nsq[:, :], 1.0, rt[:, :], op0=OP.add, op1=OP.mult)
        rec = ocp.tile([2, 512], F32, tag="z4")
        nc.vector.reciprocal(rec[:, :], den[:, :])
        if not final:
            scb = ocp.tile([2, 512], BF16, tag="z2")
            nc.vector.scalar_tensor_tensor(   # (nsq mult 1) * rec -> bf16
                scb[:, :], nsq[:, :], 1.0, rec[:, :],
                op0=OP.mult, op1=OP.mult)
            se = pt.tile([32, 512], F32, tag="t_ps")
            nc.tensor.matmul(se[:, :], lhsT=e5_sb[:, :], rhs=scb[:, :],
                             start=True, stop=True)
            nc.vector.tensor_tensor(
                vT2[:, :, :].rearrange("a p b -> a (p b)")[:, cs],
                s_all[:, cs], se[:, :], OP.mult)
        else:
            # length = nsq * r / ((1 + nsq) * (r + eps))
            nc.vector.tensor_tensor(out_sb[:, cs], num[:, :], rec[:, :],
                                    OP.mult)


def _build(cw: np.ndarray, cb: np.ndarray):
    nc = bacc.Bacc("TRN2", target_bir_lowering=False, debug=False,
                   num_devices=NCORES)

    hid_d = nc.dram_tensor("hid", [B, KC * IL], F32, kind="ExternalInput")
    w1p_d = nc.dram_tensor("w1p", [128, NQ, O, 32], BF16, kind="ExternalInput")
    w2p_d = nc.dram_tensor("w2p", [32, NP, 2 * IK], BF16, kind="ExternalInput")
    e4_d = nc.dram_tensor("e4", [32, 2], F32, kind="ExternalInput")
    e5_d = nc.dram_tensor("e5", [2, 32], BF16, kind="ExternalInput")
    out_d = nc.dram_tensor("out", [2, NP * B], F32, kind="ExternalOutput")

    with tile.TileContext(nc) as tc:
        with (
            tc.tile_pool(name="const", bufs=1) as cp,
            tc.tile_pool(name="work", bufs=1) as wp,
            tc.tile_pool(name="oc", bufs=2) as ocp,
            tc.tile_pool(name="w2s", bufs=2) as w2sp,
            tc.tile_pool(name="pt", bufs=2, space="PSUM") as pt,
            tc.tile_pool(name="pv", bufs=1, space="PSUM") as pv,
            tc.tile_pool(name="dram", bufs=2, space="DRAM") as dram_pool,
        ):
            # ---- resident weights / constants ----
            w1p_sb = cp.tile([128, NQ, O, 32], BF16, tag="w1p_sb")
            e4_sb = cp.tile([32, 2], F32, tag="e4_sb")
            e5_sb = cp.tile([2, 32], BF16, tag="e5_sb")
            nc.sync.dma_start(w1p_sb[:, :, :, :], w1p_d[:, :, :, :])
            nc.sync.dma_start(e4_sb[:, :], e4_d[:, :])
            nc.sync.dma_start(e5_sb[:, :], e5_d[:, :])

            # ---- primary capsule: 1x1 conv (8x8 linear) + squash ----
            hid = wp.tile([B, KC, IL], F32, tag="e")
            nc.sync.dma_start(hid[:, :, :], hid_d[:, :].rearrange(
                "b (k i) -> b k i", k=KC))
            xc = wp.tile([B, KC, IL], F32, tag="csm")
            for c in range(KC):
                nc.vector.tensor_scalar_mul(
                    xc[:, c, :], hid[:, 0, :], float(cw[c, 0]))
                for k in range(1, KC):
                    nc.vector.scalar_tensor_tensor(
                        xc[:, c, :], hid[:, k, :], float(cw[c, k]),
                        xc[:, c, :], op0=OP.mult, op1=OP.add)
                nc.vector.tensor_scalar_add(xc[:, c, :], xc[:, c, :],
                                            float(cb[c]))

            xsq = wp.tile([B, KC, IL], F32, tag="cT")
            nc.vector.tensor_tensor(xsq[:, :, :], xc[:, :, :], xc[:, :, :],
                                    OP.mult)
            nc.vector.tensor_tensor(xsq[:, 0:4, :], xsq[:, 0:4, :],
                                    xsq[:, 4:8, :], OP.add)
            nc.vector.tensor_tensor(xsq[:, 0:2, :], xsq[:, 0:2, :],
                                    xsq[:, 2:4, :], OP.add)
            nc.vector.tensor_tensor(xsq[:, 0, :], xsq[:, 0, :], xsq[:, 1, :],
                                    OP.add)
            nsqx = xsq[:, 0, :]                      # [128, 256] f32
            rtx = wp.tile([B, IL], F32, tag="dsum")
            nc.scalar.sqrt(rtx[:, :], nsqx)
            nc.vector.tensor_scalar_add(rtx[:, :], rtx[:, :], EPS)
            denx = wp.tile([B, IL], F32, tag="recd")
            nc.vector.scalar_tensor_tensor(
                denx[:, :], nsqx, 1.0, rtx[:, :], op0=OP.add, op1=OP.mult)
            recx = wp.tile([B, IL], F32, tag="s_sb")
            nc.vector.reciprocal(recx[:, :], denx[:, :])
            scx = wp.tile([B, IL], F32, tag="sq2_out")
            nc.vector.tensor_tensor(scx[:, :], nsqx, recx[:, :], OP.mult)

            # x in [b,(k,i)] bf16, [b,(i,k)] bf16, and [(k,i),b] layouts
            x_ki = wp.tile([B, KC, IL], BF16, tag="recb")
            nc.vector.tensor_tensor(
                x_ki[:, :, :], xc[:, :, :],
                scx[:, None, :].to_broadcast((B, KC, IL)), OP.mult)
            x2 = cp.tile([B, IL, KC], BF16, tag="x2")
            nc.vector.tensor_copy(
                x2[:, :, :], x_ki[:, :, :].rearrange("b k i -> b i k"))
            xT = cp.tile([128, NQ, B], BF16, tag="xT")
            nc.sync.dma_start_transpose(
                xT[:, :, :], x_ki[:, :, :].rearrange("b k i -> b (k i)"))

            # persistent routing state
            Bst = cp.tile([B, O, IL], BF16, tag="Bst")      # routing logits
            vT2 = cp.tile([32, NP, B], BF16, tag="vT2")     # squash(s)^T
            s_all = cp.tile([32, NP * B], F32, tag="s_all")

            # ---- iteration 0: uniform coefficients; s0 = (1/32) X W ----
            s_ps = pv.tile([32, NP, B], F32, tag="s_ps")
            for p in range(NP):
                for o2 in range(2):
                    for q in range(NQ):
                        nc.tensor.matmul(
                            s_ps[:, p, :],
                            lhsT=w1p_sb[:, q, 2 * p + o2, :],
                            rhs=xT[:, q, :],
                            start=(o2 == 0 and q == 0),
                            stop=(o2 == 1 and q == NQ - 1),
                        )
            _squash_pair(nc, wp, ocp, pt, pv, s_ps, s_all, e4_sb, e5_sb,
                         dram_pool, vT2, None, 1.0 / O, final=False)

            # ---- routing iterations 1..2 ----
            for it in (1, 2):
                for p in range(NP):
                    w2t = w2sp.tile([32, 2 * IK], BF16, tag="w2t")
                    nc.sync.dma_start(w2t[:, :], w2p_d[:, p, :])
                    for o2 in range(2):
                        o = 2 * p + o2
                        for h in range(2):
                            t_ps = pt.tile([128, 1024], F32, tag="t_ps")
                            for n in range(2):
                                sl = o2 * IK + h * 1024 + n * 512
                                nc.tensor.matmul(
                                    t_ps[:, n * 512:(n + 1) * 512],
                                    lhsT=vT2[:, p, :],
                                    rhs=w2t[:, sl:sl + 512],
                                    start=True,
                                    stop=True,
                                )
                            t_sb = ocp.tile([128, 1024], BF16, tag="t_sb")
                            nc.scalar.copy(t_sb[:, :], t_ps[:, :])
                            z = ocp.tile([128, 128, KC], BF16, tag="z")
                            nc.vector.tensor_tensor(
                                z[:, :, :],
                                x2[:, h * 128:(h + 1) * 128, :],
                                t_sb[:, :].rearrange("p (i k) -> p i k", k=KC),
                                OP.mult)
                            z4 = ocp.tile([128, 128, 4], BF16, tag="z4")
                            nc.gpsimd.tensor_tensor(
                                z4[:, :, :], z[:, :, 0:4], z[:, :, 4:8],
                                OP.add)
                            z2 = ocp.tile([128, 128, 2], BF16, tag="z2")
                            nc.gpsimd.tensor_tensor(
                                z2[:, :, :], z4[:, :, 0:2], z4[:, :, 2:4],
                                OP.add)
                            bsl = Bst[:, o, h * 128:(h + 1) * 128]
                            if it == 1:
                                nc.gpsimd.tensor_tensor(
                                    bsl, z2[:, :, 0], z2[:, :, 1], OP.add)
                            else:
                                lt = ocp.tile([128, 128], F32, tag="lt")
                                nc.gpsimd.tensor_tensor(
                                    lt[:, :], z2[:, :, 0], z2[:, :, 1], OP.add)
                                nc.gpsimd.tensor_tensor(
                                    bsl, bsl, lt[:, :], OP.add)

                # softmax over o (free axis; logits are small, skip max-sub)
                e = wp.tile([B, O, IL], BF16, tag="e")
                nc.scalar.activation(e[:, :, :], Bst[:, :, :], AF.Exp)
                d16 = wp.tile([B, 16, IL], BF16, tag="cT")
                nc.vector.tensor_tensor(d16[:, :, :], e[:, 0:16, :],
                                        e[:, 16:32, :], OP.add)
                nc.vector.tensor_tensor(d16[:, 0:8, :], d16[:, 0:8, :],
                                        d16[:, 8:16, :], OP.add)
                nc.vector.tensor_tensor(d16[:, 0:4, :], d16[:, 0:4, :],
                                        d16[:, 4:8, :], OP.add)
                nc.vector.tensor_tensor(d16[:, 0:2, :], d16[:, 0:2, :],
                                        d16[:, 2:4, :], OP.add)
                dsum = wp.tile([B, IL], F32, tag="dsum")
                nc.vector.tensor_tensor(dsum[:, :], d16[:, 0, :],
                                        d16[:, 1, :], OP.add)
                recd = wp.tile([B, IL], F32, tag="recd")
                nc.vector.reciprocal(recd[:, :], dsum[:, :])
                recb = wp.tile([B, IL], BF16, tag="recb")
                nc.vector.tensor_copy(recb[:, :], recd[:, :])
                csm = wp.tile([B, O, IL], BF16, tag="csm")
                nc.vector.tensor_tensor(
                    csm[:, :, :], e[:, :, :],
                    recb[:, None, :].to_broadcast((B, O, IL)), OP.mult)
                cT = wp.tile([128, O * 2, 128], BF16, tag="cT")
                nc.sync.dma_start_transpose(
                    cT[:, :, :], csm[:, :, :].rearrange("b o i -> b (o i)"))

                s_ps = pv.tile([32, NP, B], F32, tag="s_ps")
                for o in range(O):
                    p, o2 = divmod(o, 2)
                    yT = ocp.tile([128, KC, 2, 128], BF16, tag="yT")
                    nc.vector.tensor_tensor(
                        yT[:, :, :, :],
                        xT[:, :, :].rearrange("p (k h) b -> p k h b", k=KC),
                        cT[:, None, 2 * o:2 * o + 2, :].to_broadcast(
                            (128, KC, 2, 128)),
                        OP.mult)
                    yTq = yT[:, :, :, :].rearrange("p k h b -> p (k h) b")
                    for q in range(NQ):
                        nc.tensor.matmul(
                            s_ps[:, p, :],
                            lhsT=w1p_sb[:, q, o, :],
                            rhs=yTq[:, q, :],
                            start=(o2 == 0 and q == 0),
                            stop=(o2 == 1 and q == NQ - 1),
                        )

                if it < 2:
                    _squash_pair(nc, wp, ocp, pt, pv, s_ps, s_all, e4_sb,
                                 e5_sb, dram_pool, vT2, None, 1.0,
                                 final=False)
                else:
                    out_sb = wp.tile([2, NP * B], F32, tag="csm")
                    _squash_pair(nc, wp, ocp, pt, pv, s_ps, s_all, e4_sb,
                                 e5_sb, dram_pool, None, out_sb, 1.0,
                                 final=True)
                    nc.sync.dma_start(out_d[:, :], out_sb[:, :])

    nc.compile()
    return nc


def _host_prep(hidden, caps_w):
    """Per-core input shards + weight relayouts (pure data movement)."""
    bf = ml_dtypes.bfloat16
    hid3 = hidden.reshape(B, KC, I_FULL)
    e4 = np.zeros((32, 2), np.float32)
    e5 = np.zeros((2, 32), np.float32)
    for o2 in range(2):
        e4[o2 * 16:(o2 + 1) * 16, o2] = 1.0
        e5[o2, o2 * 16:(o2 + 1) * 16] = 1.0
    e5 = e5.astype(bf)
    maps = []
    for core in range(NCORES):
        sl = slice(core * IL, (core + 1) * IL)
        hid_loc = np.ascontiguousarray(hid3[:, :, sl]).reshape(B, KC * IL)
        wl = caps_w[:, sl]                                  # [32,256,16,8]
        # W1P [(k,i)->(p128,q16), o, (o2',d)=32] with the off-slot zeroed
        w1v = wl.transpose(3, 1, 0, 2).reshape(IK, O, D)    # [(k,i), o, d]
        w1p = np.zeros((IK, O, 32), np.float32)
        for o in range(O):
            o2 = o % 2
            w1p[:, o, o2 * 16:(o2 + 1) * 16] = w1v[:, o, :]
        w1p = np.ascontiguousarray(
            w1p.reshape(NQ, 128, O, 32).transpose(1, 0, 2, 3)).astype(bf)
        # W2P [32=(o2,d), p, o2', (i,k)] pair-block-diagonal
        wr = wl.reshape(NP, 2, IL, D, KC)                   # [p, o2, i, d, k]
        w2p = np.zeros((32, NP, 2, IL * KC), np.float32)
        for o2 in range(2):
            w2p[o2 * 16:(o2 + 1) * 16, :, o2, :] = (
                wr[:, o2].transpose(2, 0, 1, 3).reshape(D, NP, IL * KC))
        w2p = np.ascontiguousarray(w2p.reshape(32, NP, 2 * IK)).astype(bf)
        maps.append({"hid": hid_loc, "w1p": w1p, "w2p": w2p,
                     "e4": e4, "e5": e5})
    return maps


def kernel(hidden_features, conv_w, conv_b, caps_w):
    hidden = np.asarray(hidden_features, np.float32)
    cw = np.asarray(conv_w, np.float32)
    cb = np.asarray(conv_b, np.float32)
    W = np.asarray(caps_w, np.float32)

    key = (cw.tobytes(), cb.tobytes())
    if key not in _CACHE:
        _CACHE[key] = _build(cw, cb)
    nc = _CACHE[key]

    in_maps = _host_prep(hidden, W)
    res = run_bass_kernel_spmd(nc, in_maps, list(range(NCORES)))
    arr = res.results[0]["out"].reshape(2, NP, B)   # [o2, p, b]
    out = arr.transpose(2, 1, 0).reshape(B, O)      # o = 2p + o2
    return np.ascontiguousarray(out).astype(np.float32)
